# revision 1
# baseline (speedup 1.0000x reference)
"""Trainium2 Bass kernel for gnn_message_passing (nn_CMP_67181878444960).

Strategy (8-core SPMD, no collectives):
  - Host converts the edge list into two dense [V, V] count matrices
    (pos / neg).  pooled = A @ feats is then a dense matmul: each core
    computes the pooled features for its 128 nodes by streaming the full
    feats matrix [1024, 16384] through the PE (f32r, K-tiled by 128),
    spilling pooled to a DRAM scratch tensor.
  - The conv encoder is embarrassingly parallel over nodes: each core
    runs 2 residual blocks + final conv/instance-norm/relu for its 128
    nodes.  Convs are 9 shift-tap matmuls (contraction over channels on
    partitions); two nodes are packed per matmul via block-diagonal
    weights (K=96, M=96).  Boundary zero-padding is handled by clipping
    each tap's output window (PSUM has_written gives write-then-
    accumulate semantics).
"""

import functools
import sys

import numpy as np

for _p in ("/opt/trn_rl_repo",):
    if _p not in sys.path:
        sys.path.insert(0, _p)

import concourse.tile as tile  # noqa: E402
from concourse import bacc, bass_utils, mybir  # noqa: E402
from concourse.tile_rust import add_dep_helper  # noqa: E402

F32 = mybir.dt.float32
F32R = mybir.dt.float32r
BF16 = mybir.dt.bfloat16
AF = mybir.ActivationFunctionType

V, C, H = 1024, 16, 32
SP = H * H            # 1024 spatial
PW = H + 2            # padded row width (zero border)
PSP = PW * PW         # padded spatial per channel
CHW = C * SP          # 16384
C3 = 3 * C            # 48 conv channels
NCORES = 8
NPC = V // NCORES     # 128 nodes per core
EPS = 1e-5

# weight-column layout: 5 layers x 9 taps, then the two residual-conv bias rows
_LAYER_COUT = [C3, C3, C3, C3, C]          # 1a, 1b, 2a, 2b, final
_TAP_OFF = []
_off = 0
for _co in _LAYER_COUT:
    _TAP_OFF.append(_off)
    _off += 9 * 2 * _co
_BIAS1B_OFF = _off
_off += 2 * C3
_BIAS2B_OFF = _off
_off += 2 * C3
_ONES_OFF = _off
_off += 512
WCOLS = _off


def _r32(ap):
    return ap.bitcast(F32R)


def _mi(inst):
    return getattr(inst, "ins", inst)


def _interior(t):
    """AP over the H x H interior of a padded [P, PSP] tile."""
    return t[:].rearrange("p (r c) -> p r c", c=PW)[:, 1:H + 1, 1:H + 1]


def _zero_border(nc, t):
    """Zero the 1-px border of a padded tile; returns the memset insts."""
    tr = t[:].rearrange("p (r c) -> p r c", c=PW)
    return [
        nc.vector.memset(tr[:, 0:1, :], 0.0),          # top row
        nc.vector.memset(tr[:, PW - 1:PW, :], 0.0),    # bottom row
        nc.vector.memset(tr[:, 1:PW - 1, 0:1], 0.0),   # left col
        nc.vector.memset(tr[:, 1:PW - 1, PW - 1:PW], 0.0),  # right col
    ]


class _SlotGuard:
    """Explicitly order each pool slot's new first-writer after the previous
    occupant's last accessor (belt-and-braces against mis-synced reuse)."""

    def __init__(self):
        self.state = {}

    def begin(self, tag, bufs, writer_insts):
        idx, hist = self.state.setdefault(tag, [0, {}])
        prev = hist.get(idx % bufs)
        if prev is not None:
            for w in writer_insts:
                add_dep_helper(_mi(w), _mi(prev), True, "slot-reuse guard")

    def end(self, tag, bufs, last_inst):
        st = self.state.setdefault(tag, [0, {}])
        st[1][st[0] % bufs] = last_inst
        st[0] += 1


def build_kernel(tc, aps, npc, v):
    """Emit the per-core program. aps: dict of dram APs."""
    nc = tc.nc
    kt = v // 128            # K-tiles for pooling
    n_chunk = 512            # pooling column chunk
    nchunks = CHW // n_chunk
    npairs = npc // 2

    feats_pool = aps["feats_pool"]
    feats_shard = aps["feats_shard"]
    a_lhsT = aps["a_lhsT"]
    wconv = aps["wconv"]
    biases = aps["biases"]
    out = aps["out"]

    guard = _SlotGuard()
    ctx = {"guard": guard}
    build_kernel._ctx = ctx

    with (
        tc.tile_pool(name="persist", bufs=1) as persist,
        tc.tile_pool(name="psum", bufs=8, space="PSUM") as psum_pool,
    ):
        # ---- persistent SBUF state ----
        wsb = persist.tile([C3 * 2, WCOLS], BF16, tag="wsb")
        pooled = persist.tile([128, 2 * CHW], BF16, tag="pooled")
        bias_sb = persist.tile([128, 6], F32, tag="bias_sb")
        jt = persist.tile([1, 8], F32, tag="jt")
        ctx["wsb"] = wsb
        ctx["bias_sb"] = bias_sb
        ctx["ones"] = wsb[0:1, _ONES_OFF:_ONES_OFF + 512]

        nc.sync.dma_start(wsb[:], wconv[:, :])
        nc.sync.dma_start(bias_sb[:], biases[:, :])

        # ================= stage 1: pooling =================
        with (
            tc.tile_pool(name="asb", bufs=1) as asb_pool,
            tc.tile_pool(name="fstage", bufs=4) as fstage,
        ):
            a_sb = asb_pool.tile([128, kt * 2 * npc], BF16)
            nc.sync.dma_start(a_sb[:], a_lhsT[:, :])
            for cc in range(nchunks):
                fs = fstage.tile([128, kt * n_chunk], BF16, tag="fs")
                d = nc.sync.dma_start(
                    fs[:], feats_pool[cc * 128:(cc + 1) * 128, :])
                guard.begin("fs", 4, [d])
                last_mm = None
                fs_r = fs[:].rearrange("p (k n) -> p k n", k=kt)
                a_r = a_sb[:].rearrange("p (k m) -> p k m", k=kt)
                for m in range(2):
                    pp = psum_pool.tile([128, n_chunk], F32, tag="ps")
                    for k in range(kt):
                        last_mm = nc.tensor.matmul(
                            pp[:npc, :],
                            a_r[:, k, m * npc:(m + 1) * npc],
                            fs_r[:, k, :],
                            start=(k == 0),
                            stop=(k == kt - 1),
                        )
                    nc.vector.tensor_copy(
                        pooled[:npc, m * CHW + cc * n_chunk:
                               m * CHW + (cc + 1) * n_chunk],
                        pp[:npc, :],
                    )
                guard.end("fs", 4, last_mm)

        # ================= stage 2: conv encoder =================
        with (
            tc.tile_pool(name="stg", bufs=4) as stpool,
            tc.tile_pool(name="xt", bufs=5) as xpool,
            tc.tile_pool(name="ht", bufs=5) as hpool,
            tc.tile_pool(name="ot", bufs=3) as opool,
            tc.tile_pool(name="nrm", bufs=6) as nrm,
        ):
            for p in range(npairs):
                st = stpool.tile([2 * C3, SP], BF16, tag="stg")
                # assemble x = [feats | pooled_pos | pooled_neg] per node
                wrts = []
                for n in range(2):
                    wrts.append(nc.gpsimd.dma_start(
                        st[48 * n:48 * n + 16, :],
                        feats_shard[2 * p + n:2 * p + n + 1, :].rearrange(
                            "o (c s) -> (o c) s", c=C),
                    ))
                    for m in range(2):
                        wrts.append(nc.gpsimd.dma_start(
                            st[48 * n + 16 * (m + 1):48 * n + 16 * (m + 2), :],
                            pooled[2 * p + n:2 * p + n + 1,
                                   m * CHW:(m + 1) * CHW],
                        ))
                guard.begin("stg", 4, wrts)

                x = xpool.tile([2 * C3, PSP], BF16, tag="x")
                bz = _zero_border(nc, x)
                guard.begin("x", 5, bz)
                cpx = nc.vector.tensor_copy(
                    _interior(x),
                    st[:].rearrange("p (r c) -> p r c", c=H),
                )
                guard.end("stg", 4, cpx)

                # residual block 1
                h = hpool.tile([2 * C3, PSP], BF16, tag="h")
                hb = _zero_border(nc, h)
                guard.begin("h", 5, hb)
                hw = _conv(tc, psum_pool, x, h, 0, relu=True, bias_col=0)
                hl = _conv(tc, psum_pool, h, x, 1, resid=True,
                           bias_col=4, tmp_pool=stpool)
                guard.end("h", 5, hl[-1])
                # residual block 2
                h2 = hpool.tile([2 * C3, PSP], BF16, tag="h")
                hb2 = _zero_border(nc, h2)
                guard.begin("h", 5, hb2)
                hw2 = _conv(tc, psum_pool, x, h2, 2, relu=True, bias_col=1)
                hl2 = _conv(tc, psum_pool, h2, x, 3, resid=True,
                            bias_col=5, tmp_pool=stpool)
                guard.end("h", 5, hl2[-1])
                # final conv + instance norm + relu
                ot = opool.tile([2 * C, SP], F32, tag="ot")
                ow = _conv(tc, psum_pool, x, ot, 4, final=True, bias_col=2)
                guard.begin("ot", 3, ow)
                guard.end("x", 5, ow[-1])

                stats = nrm.tile([2 * C, 12], F32, tag="stats")
                mv = nrm.tile([2 * C, 2], F32, tag="mv")
                sc = nrm.tile([2 * C, 3], F32, tag="sc")
                nc.vector.bn_stats(stats[:, 0:6], ot[:, 0:512])
                nc.vector.bn_stats(stats[:, 6:12], ot[:, 512:1024])
                nc.vector.bn_aggr(mv[:], stats[:])
                # sc0 = sqrt(var+eps); sc1 = 1/sc0; sc2 = -mean/sc0
                nc.scalar.activation(sc[:, 0:1], mv[:, 1:2], AF.Sqrt,
                                     bias=bias_sb[:2 * C, 3:4])
                nc.vector.reciprocal(sc[:, 1:2], sc[:, 0:1])
                nc.vector.tensor_scalar(
                    sc[:, 2:3], mv[:, 0:1], sc[:, 1:2], -1.0,
                    op0=mybir.AluOpType.mult, op1=mybir.AluOpType.mult,
                )
                fin = opool.tile([2 * C, SP], F32, tag="fin")
                ap_i = nc.scalar.activation(
                    fin[:], ot[:], AF.Relu, bias=sc[:, 2:3], scale=sc[:, 1:2]
                )
                guard.begin("fin", 3, [ap_i])
                guard.end("ot", 3, ap_i)
                od = nc.sync.dma_start(out[2 * p:2 * p + 2, :], fin[:])
                guard.end("fin", 3, od)


def _conv(tc, psum_pool, xin, xout, layer, relu=False, resid=False,
          final=False, bias_col=None, bias_off=None, tmp_pool=None):
    """One 3x3 'SAME' conv for a node pair.

    xin:  [96, 1024] (node, ch) x spatial
    xout: relu  -> write relu(conv+bias) into xout (dense)
          resid -> xout += conv + bias (bias via K=1 ones-matmul)
          final -> copy conv+bias into xout (2*C partitions)
    Returns the per-halftile tail instructions (ACT/DVE).
    """
    nc = tc.nc
    ctx = build_kernel._ctx
    wsb, bias_sb, ones_t = ctx["wsb"], ctx["bias_sb"], ctx["ones"]

    cout = _LAYER_COUT[layer]
    m = 2 * cout
    xr = xin[:].rearrange("p (r c) -> p r c", c=PW)
    if not final:
        outr = xout[:].rearrange("p (r c) -> p r c", c=PW)

    taps = [(dy, dx) for dy in (-1, 0, 1) for dx in (-1, 0, 1)]

    tails = []
    for nt in range(2):
        r0 = nt * 16
        pp = psum_pool.tile([128, 512], F32, tag="ps")
        ppr = pp[:].rearrange("p (r c) -> p r c", c=H)
        first = True
        for i, (dy, dx) in enumerate(taps):
            # out rows r0..r0+16, cols 0..32 read padded window
            ky, kx = dy + 1, dx + 1
            woff = _TAP_OFF[layer] + (ky * 3 + kx) * m
            nc.tensor.matmul(
                pp[:m, :512],
                wsb[0:2 * C3, woff:woff + m],
                xr[0:2 * C3, r0 + ky:r0 + ky + 16, kx:kx + H],
                start=first, stop=(i == len(taps) - 1),
                skip_group_check=True,
            )
            first = False
        if relu:
            t = nc.scalar.activation(
                outr[:, 1 + r0:1 + r0 + 16, 1:1 + H], ppr[:m],
                AF.Relu, bias=bias_sb[:m, bias_col:bias_col + 1],
            )
        elif final:
            t = nc.scalar.activation(
                xout[:, nt * 512:(nt + 1) * 512], pp[:m, :],
                AF.Identity, bias=bias_sb[:m, bias_col:bias_col + 1],
            )
        else:  # resid: xout += conv + bias (ACT adds bias, DVE adds x)
            tmp = tmp_pool.tile([2 * C3, 512], BF16, tag="tmp")
            nc.scalar.activation(
                tmp[:m, :], pp[:m, :],
                AF.Identity, bias=bias_sb[:m, bias_col:bias_col + 1],
            )
            t = nc.vector.tensor_add(
                outr[:, 1 + r0:1 + r0 + 16, 1:1 + H],
                outr[:, 1 + r0:1 + r0 + 16, 1:1 + H],
                tmp[:m, :].rearrange("p (r c) -> p r c", c=H),
            )
        tails.append(t)
    return tails


# ======================= host side =======================

def _prep_weights(w_list, b_list):
    """Pack conv weights into the [96, WCOLS] f32 lhsT array."""
    wsb = np.zeros((2 * C3, WCOLS), np.float32)
    for layer, (w, b) in enumerate(zip(w_list, b_list)):
        co = _LAYER_COUT[layer]
        for ky in range(3):
            for kx in range(3):
                lt = np.ascontiguousarray(w[:, :, ky, kx].T)  # [C_in, C_out]
                off = _TAP_OFF[layer] + (ky * 3 + kx) * 2 * co
                wsb[0:C3, off:off + co] = lt
                wsb[C3:2 * C3, off + co:off + 2 * co] = lt
    # residual-conv biases live on partition 0 as K=1 lhsT rows
    wsb[0, _BIAS1B_OFF:_BIAS1B_OFF + 2 * C3] = np.tile(b_list[1], 2)
    wsb[0, _BIAS2B_OFF:_BIAS2B_OFF + 2 * C3] = np.tile(b_list[3], 2)
    wsb[0, _ONES_OFF:_ONES_OFF + 512] = 1.0
    import ml_dtypes
    return wsb.astype(ml_dtypes.bfloat16)


def _prep_biases(b1a, b2a, bf, b1b, b2b):
    bias = np.zeros((128, 6), np.float32)
    bias[0:96, 0] = np.tile(b1a, 2)
    bias[0:96, 1] = np.tile(b2a, 2)
    bias[0:2 * C, 2] = np.tile(bf, 2)
    bias[:, 3] = EPS
    bias[0:96, 4] = np.tile(b1b, 2)
    bias[0:96, 5] = np.tile(b2b, 2)
    return bias


def _build_adjacency(edges, v):
    src, lab, dst = edges[:, 0], edges[:, 1], edges[:, 2]
    a = np.zeros((2, v, v), np.float32)
    for mi, mask in enumerate((lab > 0, lab < 0)):
        s, d = src[mask], dst[mask]
        np.add.at(a[mi], (d, s), 1.0)
        np.add.at(a[mi], (s, d), 1.0)
    return a


@functools.lru_cache(maxsize=2)
def _build_module(npc, v, ncores):
    nc = bacc.Bacc(
        "TRN2", target_bir_lowering=False, debug=False,
        enable_asserts=False, num_devices=ncores,
    )
    aps = {
        "feats_pool": nc.dram_tensor("feats_pool", [(CHW // 512) * 128,
                                     (v // 128) * 512], BF16,
                                     kind="ExternalInput").ap(),
        "feats_shard": nc.dram_tensor("feats_shard", [npc, CHW], BF16,
                                      kind="ExternalInput").ap(),
        "a_lhsT": nc.dram_tensor("a_lhsT", [128, (v // 128) * 2 * npc], BF16,
                                 kind="ExternalInput").ap(),
        "wconv": nc.dram_tensor("wconv", [2 * C3, WCOLS], BF16,
                                kind="ExternalInput").ap(),
        "biases": nc.dram_tensor("biases", [128, 6], F32,
                                 kind="ExternalInput").ap(),
        "out": nc.dram_tensor("out", [npc, CHW], F32,
                              kind="ExternalOutput").ap(),
    }
    with tile.TileContext(nc) as tc:
        build_kernel(tc, aps, npc, v)
    nc.compile()
    return nc


def make_in_maps(feats, edges, w1a, b1a, w1b, b1b, w2a, b2a, w2b, b2b,
                 wf, bf, ncores=NCORES, v=V):
    feats = np.ascontiguousarray(np.asarray(feats, np.float32)).reshape(v, CHW)
    edges = np.asarray(edges)
    npc = v // ncores
    a = _build_adjacency(edges, v)
    wsb = _prep_weights(
        [np.asarray(w) for w in (w1a, w1b, w2a, w2b, wf)],
        [np.asarray(b) for b in (b1a, b1b, b2a, b2b, bf)],
    )
    bias = _prep_biases(np.asarray(b1a), np.asarray(b2a), np.asarray(bf),
                    np.asarray(b1b), np.asarray(b2b))
    in_maps = []
    for i in range(ncores):
        rows = slice(i * npc, (i + 1) * npc)
        a_sel = np.concatenate([a[0, rows], a[1, rows]], axis=0)  # [2*npc, V]
        import ml_dtypes
        kt = v // 128
        nch = CHW // 512
        fp = feats.reshape(kt, 128, nch, 512).transpose(2, 1, 0, 3)
        fp = np.ascontiguousarray(fp).reshape(nch * 128, kt * 512)
        alt = a_sel.T.reshape(kt, 128, 2 * npc).transpose(1, 0, 2)
        alt = np.ascontiguousarray(alt).reshape(128, kt * 2 * npc)
        in_maps.append({
            "feats_pool": fp.astype(ml_dtypes.bfloat16),
            "feats_shard": np.ascontiguousarray(feats[rows]).astype(
                ml_dtypes.bfloat16),
            "a_lhsT": alt.astype(ml_dtypes.bfloat16),
            "wconv": wsb,
            "biases": bias,
        })
    return in_maps


def run(inputs, trace=False):
    in_maps = make_in_maps(**inputs)
    nc = _build_module(NPC, V, NCORES)
    res = bass_utils.run_bass_kernel_spmd(
        nc, in_maps, core_ids=list(range(NCORES)), trace=trace,
    )
    out = np.concatenate(
        [res.results[i]["out"] for i in range(NCORES)], axis=0
    ).reshape(V, C, H, H)
    return out, res


def kernel(**inputs):
    out, _ = run(inputs, trace=False)
    return out



# revision 9
# speedup vs baseline: 1.9041x; 1.9041x over previous
"""Trainium2 Bass kernel for gnn_message_passing (nn_CMP_67181878444960).

Strategy (8-core SPMD, no collectives):
  - Host converts the edge list into two dense [V, V] count matrices
    (pos / neg).  pooled = A @ feats is a dense matmul: each core
    computes the pooled features for its 128 nodes by streaming the full
    feats matrix [1024, 16384] through the PE (bf16, K-tiled by 128).
  - The conv encoder is embarrassingly parallel over nodes.  Convs are
    9 shift-tap matmuls (contraction over channels on partitions).  The
    PE array is run in 64x64 tiling mode: four 48x48 tap-matmuls (four
    nodes) execute CONCURRENTLY on the four array tiles, doubling
    throughput vs. 128x128 block-diagonal node pairing.  Nodes A/B sit
    on SBUF partition groups 0-47 / 64-111; nodes C/D at a second
    free-dim slot of the same tiles.  The final 48->16 conv uses 64x32
    tiling (8 tiles) across two node groups at once.
"""

import functools
import sys

import numpy as np

for _p in ("/opt/trn_rl_repo",):
    if _p not in sys.path:
        sys.path.insert(0, _p)

import concourse.tile as tile  # noqa: E402
from concourse import bacc, bass_utils, mybir  # noqa: E402
from concourse.tile_rust import add_dep_helper  # noqa: E402

F32 = mybir.dt.float32
BF16 = mybir.dt.bfloat16
AF = mybir.ActivationFunctionType

V, C, H = 1024, 16, 32
SP = H * H            # 1024 spatial
PW = H + 2            # padded row width (zero border)
PSP = PW * PW         # padded spatial per channel
CHW = C * SP          # 16384
C3 = 3 * C            # 48 conv channels
NCORES = 8
NPC = V // NCORES     # 128 nodes per core
EPS = 1e-5
TAPS = [(ky, kx) for ky in range(3) for kx in range(3)]

# weight-column layout: per layer a block of 9 taps x cout
_LAYER_COUT = [C3, C3, C3, C3, C]          # 1a, 1b, 2a, 2b, final
_LAYER_OFF = []
_off = 0
for _co in _LAYER_COUT:
    _LAYER_OFF.append(_off)
    _off += 9 * _co
WCOLS = _off

# partition blocks for the 48-channel layers and the 16-channel final
_B48 = ((0, 48), (64, 112))
_B16 = ((0, 16), (32, 48), (64, 80), (96, 112))


def _mi(inst):
    return getattr(inst, "ins", inst)


class _SlotGuard:
    """Explicitly order each pool slot's new first-writer after the previous
    occupant's last accessor (belt-and-braces against mis-synced reuse)."""

    def __init__(self):
        self.state = {}

    def begin(self, tag, bufs, writer_insts):
        st = self.state.setdefault(tag, [0, 0, {}])
        prev = st[2].get(st[0] % bufs)
        if prev is not None:
            for w in writer_insts:
                add_dep_helper(_mi(w), _mi(prev), True, "slot-reuse guard")
        st[0] += 1

    def end(self, tag, bufs, last_inst):
        st = self.state.setdefault(tag, [0, 0, {}])
        st[2][st[1] % bufs] = last_inst
        st[1] += 1


def _pad4(t):
    """[p, (slot, padded r, padded c)] view of a [128, 2*PSP] tile."""
    return t[:].rearrange("p (s r c) -> p s r c", s=2, c=PW)


def _zero_borders(nc, t):
    tr = _pad4(t)
    ins = []
    for s in range(2):
        ins.append(nc.vector.memset(tr[:, s, 0:1, :], 0.0))
        ins.append(nc.vector.memset(tr[:, s, PW - 1:PW, :], 0.0))
        ins.append(nc.vector.memset(tr[:, s, 1:PW - 1, 0:1], 0.0))
        ins.append(nc.vector.memset(tr[:, s, 1:PW - 1, PW - 1:PW], 0.0))
    return ins


def build_kernel(tc, aps, npc, v):
    nc = tc.nc
    kt = v // 128            # K-tiles for pooling
    n_chunk = 512            # pooling column chunk
    nchunks = CHW // n_chunk
    ngroups = npc // 4       # 4-node groups

    feats_pool = aps["feats_pool"]
    feats_shard = aps["feats_shard"]
    a_lhsT = aps["a_lhsT"]
    wconv = aps["wconv"]
    biases = aps["biases"]
    out = aps["out"]

    guard = _SlotGuard()

    with (
        tc.tile_pool(name="persist", bufs=1) as persist,
        tc.tile_pool(name="psum", bufs=4, space="PSUM") as psum_pool,
    ):
        # ---- persistent SBUF state ----
        wsb = persist.tile([128, WCOLS], BF16, tag="wsb")
        pooled = persist.tile([128, 2 * CHW], BF16, tag="pooled")
        bias_sb = persist.tile([128, 6], F32, tag="bias_sb")
        nc.sync.dma_start(wsb[:], wconv[:, :])
        nc.sync.dma_start(bias_sb[:], biases[:, :])

        # ================= stage 1: pooling =================
        with (
            tc.tile_pool(name="asb", bufs=1) as asb_pool,
            tc.tile_pool(name="fstage", bufs=4) as fstage,
        ):
            a_sb = asb_pool.tile([128, kt * 2 * npc], BF16)
            nc.sync.dma_start(a_sb[:], a_lhsT[:, :])
            for cc in range(nchunks):
                fs = fstage.tile([128, kt * n_chunk], BF16, tag="fs")
                d = nc.sync.dma_start(
                    fs[:], feats_pool[cc * 128:(cc + 1) * 128, :])
                guard.begin("fs", 4, [d])
                last_mm = None
                fs_r = fs[:].rearrange("p (k n) -> p k n", k=kt)
                a_r = a_sb[:].rearrange("p (k m) -> p k m", k=kt)
                for m in range(2):
                    pp = psum_pool.tile([128, n_chunk], F32,
                                        tag=("psP" if m == 0 else "psQ"))
                    for k in range(kt):
                        last_mm = nc.tensor.matmul(
                            pp[:npc, :],
                            a_r[:, k, m * npc:(m + 1) * npc],
                            fs_r[:, k, :],
                            start=(k == 0),
                            stop=(k == kt - 1),
                        )
                    nc.vector.tensor_copy(
                        pooled[:npc, m * CHW + cc * n_chunk:
                               m * CHW + (cc + 1) * n_chunk],
                        pp[:npc, :],
                    )
                guard.end("fs", 4, last_mm)

        # ================= stage 2: conv encoder =================
        # group g = nodes 4g..4g+3: (rg, slot) -> node:
        #   (0, s0)=4g+0  (64, s0)=4g+1  (0, s1)=4g+2  (64, s1)=4g+3
        with (
            tc.tile_pool(name="stg", bufs=3) as stpool,
            tc.tile_pool(name="xt", bufs=5) as xpool,
            tc.tile_pool(name="ht", bufs=4) as hpool,
            tc.tile_pool(name="tmp", bufs=4) as tmppool,
            tc.tile_pool(name="ot", bufs=3) as opool,
            tc.tile_pool(name="fin", bufs=3) as finpool,
            tc.tile_pool(name="nrm", bufs=4) as nrm,
        ):
            ctx = {
                "nc": nc, "wsb": wsb, "bias_sb": bias_sb,
                "psum": psum_pool, "tmp": tmppool, "guard": guard,
            }

            def assemble(g):
                """Stage nodes 4g..4g+3 into a padded x tile."""
                st = stpool.tile([128, 2 * SP], BF16, tag="stg")
                wrts = []
                for rb, s, n in ((0, 0, 4 * g), (64, 0, 4 * g + 1),
                                 (0, 1, 4 * g + 2), (64, 1, 4 * g + 3)):
                    dst0 = st[rb:rb + C, s * SP:(s + 1) * SP]
                    wrts.append(nc.gpsimd.dma_start(
                        dst0,
                        feats_shard[n:n + 1, :].rearrange(
                            "o (c z) -> (o c) z", c=C),
                    ))
                    for m in range(2):
                        wrts.append(nc.gpsimd.dma_start(
                            st[rb + C * (m + 1):rb + C * (m + 2),
                               s * SP:(s + 1) * SP],
                            pooled[n:n + 1, m * CHW:(m + 1) * CHW],
                        ))
                guard.begin("stg", 3, wrts)

                x = xpool.tile([128, 2 * PSP], BF16, tag="x")
                bz = _zero_borders(nc, x)
                guard.begin("x", 5, bz)
                xr = _pad4(x)
                str_ = st[:].rearrange("p (s r c) -> p s r c", s=2, c=H)
                cps = []
                for s in range(2):
                    cps.append(nc.vector.tensor_copy(
                        xr[0:112, s, 1:1 + H, 1:1 + H], str_[0:112, s]))
                guard.end("stg", 3, cps[-1])
                return x

            def emit48(src_t, layer, evac):
                """One 48->48 3x3 conv for 4 nodes on 64x64 PE tiles.
                evac(hf, pP, pQ) -> last instructions."""
                base = _LAYER_OFF[layer]
                srcr = _pad4(src_t)
                lasts = []
                for hf in range(2):
                    pP = psum_pool.tile([128, 512], F32, tag="psP")
                    pQ = psum_pool.tile([128, 512], F32, tag="psQ")
                    for t, (ky, kx) in enumerate(TAPS):
                        st_, sp_ = (t == 0), (t == 8)
                        wo = base + t * C3
                        for rb, s, pp, ob in ((0, 0, pP, 0), (64, 0, pQ, 0),
                                              (0, 1, pP, 64), (64, 1, pQ, 64)):
                            nc.tensor.matmul(
                                pp[ob:ob + C3, :],
                                wsb[rb:rb + C3, wo:wo + C3],
                                srcr[rb:rb + C3, s,
                                     hf * 16 + ky:hf * 16 + ky + 16,
                                     kx:kx + H],
                                start=st_, stop=sp_,
                                skip_group_check=True,
                                tile_position=(rb, ob),
                            )
                    lasts.extend(evac(hf, pP, pQ))
                return lasts

            def relu_evac(dst_t, bias_col):
                dr = _pad4(dst_t)

                def evac(hf, pP, pQ):
                    res = []
                    for s, pp in ((0, pP), (1, pQ)):
                        res.append(nc.scalar.activation(
                            dr[0:112, s, 1 + hf * 16:1 + hf * 16 + 16,
                               1:1 + H],
                            pp[0:112, :], AF.Relu,
                            bias=bias_sb[0:112, bias_col:bias_col + 1],
                        ))
                    return res
                return evac

            def resid_evac(x_t, bias_col):
                xr = _pad4(x_t)

                def evac(hf, pP, pQ):
                    res = []
                    for s, pp in ((0, pP), (1, pQ)):
                        tmp = tmppool.tile([128, 512], BF16, tag="tmp")
                        a = nc.scalar.activation(
                            tmp[0:112, :], pp[0:112, :], AF.Identity,
                            bias=bias_sb[0:112, bias_col:bias_col + 1],
                        )
                        guard.begin("tmp", 4, [a])
                        win = xr[0:112, s, 1 + hf * 16:1 + hf * 16 + 16,
                                 1:1 + H]
                        t = nc.vector.tensor_add(
                            win, win,
                            tmp[0:112, :].rearrange("p (r c) -> p r c", c=H))
                        guard.end("tmp", 4, t)
                        res.append(t)
                    return res
                return evac

            def emit_final(x0, x1, g0, g1):
                """48->16 conv for groups g0,g1 on 64x32 PE tiles (8 nodes),
                then instance-norm + relu + output DMA."""
                base = _LAYER_OFF[4]
                ot1 = opool.tile([128, SP], F32, tag="ot")
                ot2 = opool.tile([128, SP], F32, tag="ot")
                evs = []
                last_mm = None
                for hf in range(2):
                    pP = psum_pool.tile([128, 512], F32, tag="psP")
                    pQ = psum_pool.tile([128, 512], F32, tag="psQ")
                    tiles = []
                    for gi, xt in ((0, x0), (1, x1)):
                        for rb, s, pp, cp in (
                                (0, 0, pP, 64 * gi),
                                (64, 0, pQ, 64 * gi),
                                (0, 1, pP, 64 * gi + 32),
                                (64, 1, pQ, 64 * gi + 32)):
                            tiles.append((xt, rb, s, pp, cp))
                    for t, (ky, kx) in enumerate(TAPS):
                        st_, sp_ = (t == 0), (t == 8)
                        wo = base + t * C
                        for xt, rb, s, pp, cp in tiles:
                            last_mm = nc.tensor.matmul(
                                pp[cp:cp + C, :],
                                wsb[rb:rb + C3, wo:wo + C],
                                _pad4(xt)[rb:rb + C3, s,
                                          hf * 16 + ky:hf * 16 + ky + 16,
                                          kx:kx + H],
                                start=st_, stop=sp_,
                                skip_group_check=True,
                                tile_position=(rb, cp),
                            )
                    for ot, pp in ((ot1, pP), (ot2, pQ)):
                        e = nc.scalar.activation(
                            ot[0:112, hf * 512:(hf + 1) * 512],
                            pp[0:112, :], AF.Identity,
                            bias=bias_sb[0:112, 2:3])
                        evs.append(e)
                # instance norm + relu + store; ot1 holds nodes
                # (4g0, 4g0+2, 4g1, 4g1+2), ot2 the odd ones.
                for oi, (ot, n_off) in enumerate(((ot1, 0), (ot2, 1))):
                    guard.begin("ot", 3, [evs[oi], evs[oi + 2]])
                    stats = nrm.tile([128, 12], F32, tag="stats")
                    mv = nrm.tile([128, 2], F32, tag="mv")
                    sc = nrm.tile([128, 3], F32, tag="sc")
                    nc.vector.bn_stats(stats[0:112, 0:6], ot[0:112, 0:512])
                    nc.vector.bn_stats(stats[0:112, 6:12], ot[0:112, 512:1024])
                    nc.vector.bn_aggr(mv[0:112, :], stats[0:112, :])
                    nc.scalar.activation(sc[0:112, 0:1], mv[0:112, 1:2],
                                         AF.Sqrt, bias=bias_sb[0:112, 3:4])
                    nc.vector.reciprocal(sc[0:112, 1:2], sc[0:112, 0:1])
                    nc.vector.tensor_scalar(
                        sc[0:112, 2:3], mv[0:112, 0:1], sc[0:112, 1:2], -1.0,
                        op0=mybir.AluOpType.mult, op1=mybir.AluOpType.mult,
                    )
                    fin = finpool.tile([128, SP], F32, tag="fin")
                    ap_i = nc.scalar.activation(
                        fin[0:112, :], ot[0:112, :], AF.Relu,
                        bias=sc[0:112, 2:3], scale=sc[0:112, 1:2])
                    guard.begin("fin", 3, [ap_i])
                    guard.end("ot", 3, ap_i)
                    dmas = []
                    for (p0, p1), n in zip(
                            _B16, (4 * g0, 4 * g0 + 2, 4 * g1, 4 * g1 + 2)):
                        dmas.append(nc.sync.dma_start(
                            out[n + n_off:n + n_off + 1, :].rearrange(
                                "o (c z) -> (o c) z", c=C),
                            fin[p0:p1, :]))
                    guard.end("fin", 3, dmas[-1])
                return last_mm

            # ---- main conv loop: groups in pairs for PE pipelining ----
            for gp in range(ngroups // 2):
                g0, g1 = 2 * gp, 2 * gp + 1
                xs = {}
                hs = {}
                for g in (g0, g1):
                    xs[g] = assemble(g)
                for g in (g0, g1):
                    h = hpool.tile([128, 2 * PSP], BF16, tag="h")
                    bz = _zero_borders(nc, h)
                    guard.begin("h", 4, bz)
                    hs[g] = h
                    emit48(xs[g], 0, relu_evac(h, 0))
                for g in (g0, g1):
                    lasts = emit48(hs[g], 1, resid_evac(xs[g], 4))
                    guard.end("h", 4, lasts[-1])
                for g in (g0, g1):
                    h2 = hpool.tile([128, 2 * PSP], BF16, tag="h")
                    bz = _zero_borders(nc, h2)
                    guard.begin("h", 4, bz)
                    hs[g] = h2
                    emit48(xs[g], 2, relu_evac(h2, 1))
                for g in (g0, g1):
                    lasts = emit48(hs[g], 3, resid_evac(xs[g], 5))
                    guard.end("h", 4, lasts[-1])
                last_mm = emit_final(xs[g0], xs[g1], g0, g1)
                # x tiles die after the final conv's matmul reads
                guard.end("x", 5, last_mm)
                guard.end("x", 5, last_mm)


# ======================= host side =======================

def _prep_weights(w_list):
    """Pack conv weights into the [128, WCOLS] bf16 lhsT array (two
    partition-group copies at rows 0-47 and 64-111)."""
    wsb = np.zeros((128, WCOLS), np.float32)
    for layer, w in enumerate(w_list):
        co = _LAYER_COUT[layer]
        base = _LAYER_OFF[layer]
        for t, (ky, kx) in enumerate(TAPS):
            lt = np.ascontiguousarray(w[:, :, ky, kx].T)  # [C_in, C_out]
            off = base + t * co
            wsb[0:C3, off:off + co] = lt
            wsb[64:64 + C3, off:off + co] = lt
    import ml_dtypes
    return wsb.astype(ml_dtypes.bfloat16)


def _prep_biases(b1a, b2a, bf, b1b, b2b):
    bias = np.zeros((128, 6), np.float32)
    for p0, p1 in _B48:
        bias[p0:p1, 0] = b1a
        bias[p0:p1, 1] = b2a
        bias[p0:p1, 4] = b1b
        bias[p0:p1, 5] = b2b
    for p0, p1 in _B16:
        bias[p0:p1, 2] = bf
    bias[:, 3] = EPS
    return bias


def _build_adjacency(edges, v):
    src, lab, dst = edges[:, 0], edges[:, 1], edges[:, 2]
    a = np.zeros((2, v, v), np.float32)
    for mi, mask in enumerate((lab > 0, lab < 0)):
        s, d = src[mask], dst[mask]
        np.add.at(a[mi], (d, s), 1.0)
        np.add.at(a[mi], (s, d), 1.0)
    return a


@functools.lru_cache(maxsize=2)
def _build_module(npc, v, ncores):
    nc = bacc.Bacc(
        "TRN2", target_bir_lowering=False, debug=False,
        enable_asserts=False, num_devices=ncores,
    )
    aps = {
        "feats_pool": nc.dram_tensor("feats_pool", [(CHW // 512) * 128,
                                     (v // 128) * 512], BF16,
                                     kind="ExternalInput").ap(),
        "feats_shard": nc.dram_tensor("feats_shard", [npc, CHW], BF16,
                                      kind="ExternalInput").ap(),
        "a_lhsT": nc.dram_tensor("a_lhsT", [128, (v // 128) * 2 * npc], BF16,
                                 kind="ExternalInput").ap(),
        "wconv": nc.dram_tensor("wconv", [128, WCOLS], BF16,
                                kind="ExternalInput").ap(),
        "biases": nc.dram_tensor("biases", [128, 6], F32,
                                 kind="ExternalInput").ap(),
        "out": nc.dram_tensor("out", [npc, CHW], F32,
                              kind="ExternalOutput").ap(),
    }
    with tile.TileContext(nc) as tc:
        build_kernel(tc, aps, npc, v)
    nc.compile()
    return nc


def make_in_maps(feats, edges, w1a, b1a, w1b, b1b, w2a, b2a, w2b, b2b,
                 wf, bf, ncores=NCORES, v=V):
    feats = np.ascontiguousarray(np.asarray(feats, np.float32)).reshape(v, CHW)
    edges = np.asarray(edges)
    npc = v // ncores
    a = _build_adjacency(edges, v)
    wsb = _prep_weights([np.asarray(w) for w in (w1a, w1b, w2a, w2b, wf)])
    bias = _prep_biases(np.asarray(b1a), np.asarray(b2a), np.asarray(bf),
                        np.asarray(b1b), np.asarray(b2b))
    in_maps = []
    for i in range(ncores):
        rows = slice(i * npc, (i + 1) * npc)
        a_sel = np.concatenate([a[0, rows], a[1, rows]], axis=0)  # [2*npc, V]
        import ml_dtypes
        kt = v // 128
        nch = CHW // 512
        fp = feats.reshape(kt, 128, nch, 512).transpose(2, 1, 0, 3)
        fp = np.ascontiguousarray(fp).reshape(nch * 128, kt * 512)
        alt = a_sel.T.reshape(kt, 128, 2 * npc).transpose(1, 0, 2)
        alt = np.ascontiguousarray(alt).reshape(128, kt * 2 * npc)
        in_maps.append({
            "feats_pool": fp.astype(ml_dtypes.bfloat16),
            "feats_shard": np.ascontiguousarray(feats[rows]).astype(
                ml_dtypes.bfloat16),
            "a_lhsT": alt.astype(ml_dtypes.bfloat16),
            "wconv": wsb,
            "biases": bias,
        })
    return in_maps


def run(inputs, trace=False):
    in_maps = make_in_maps(**inputs)
    nc = _build_module(NPC, V, NCORES)
    res = bass_utils.run_bass_kernel_spmd(
        nc, in_maps, core_ids=list(range(NCORES)), trace=trace,
    )
    out = np.concatenate(
        [res.results[i]["out"] for i in range(NCORES)], axis=0
    ).reshape(V, C, H, H)
    return out, res


def kernel(**inputs):
    out, _ = run(inputs, trace=False)
    return out


# revision 13
# speedup vs baseline: 1.9193x; 1.0080x over previous
"""Trainium2 Bass kernel for gnn_message_passing (nn_CMP_67181878444960).

Strategy (8-core SPMD, no collectives):
  - Host converts the edge list into two dense [V, V] count matrices
    (pos / neg).  pooled = A @ feats is a dense matmul: each core
    computes the pooled features for its 128 nodes by streaming the full
    feats matrix [1024, 16384] through the PE (bf16, K-tiled by 128).
  - The conv encoder is embarrassingly parallel over nodes.  Convs are
    9 shift-tap matmuls (contraction over channels on partitions).  The
    PE array is run in 64x64 tiling mode: four 48x48 tap-matmuls (four
    nodes) execute CONCURRENTLY on the four array tiles, doubling
    throughput vs. 128x128 block-diagonal node pairing.  Nodes A/B sit
    on SBUF partition groups 0-47 / 64-111; nodes C/D at a second
    free-dim slot of the same tiles.  The final 48->16 conv uses 64x32
    tiling (8 tiles) across two node groups at once.
"""

import functools
import sys

import numpy as np

for _p in ("/opt/trn_rl_repo",):
    if _p not in sys.path:
        sys.path.insert(0, _p)

import concourse.tile as tile  # noqa: E402
from concourse import bacc, bass_utils, mybir  # noqa: E402
from concourse.tile_rust import add_dep_helper  # noqa: E402

F32 = mybir.dt.float32
BF16 = mybir.dt.bfloat16
AF = mybir.ActivationFunctionType

V, C, H = 1024, 16, 32
SP = H * H            # 1024 spatial
PW = H + 2            # padded row width (zero border)
PSP = PW * PW         # padded spatial per channel
CHW = C * SP          # 16384
C3 = 3 * C            # 48 conv channels
NCORES = 8
NPC = V // NCORES     # 128 nodes per core
EPS = 1e-5
TAPS = [(ky, kx) for ky in range(3) for kx in range(3)]

# weight-column layout: per layer a block of 9 taps x cout
_LAYER_COUT = [C3, C3, C3, C3, C]          # 1a, 1b, 2a, 2b, final
_LAYER_OFF = []
_off = 0
for _co in _LAYER_COUT:
    _LAYER_OFF.append(_off)
    _off += 9 * _co
WCOLS = _off

# partition blocks for the 48-channel layers and the 16-channel final
_B48 = ((0, 48), (64, 112))
_B16 = ((0, 16), (32, 48), (64, 80), (96, 112))


def _mi(inst):
    return getattr(inst, "ins", inst)


class _SlotGuard:
    """Explicitly order each pool slot's new first-writer after the previous
    occupant's last accessor (belt-and-braces against mis-synced reuse)."""

    def __init__(self):
        self.state = {}

    def begin(self, tag, bufs, writer_insts):
        st = self.state.setdefault(tag, [0, 0, {}])
        prev = st[2].get(st[0] % bufs)
        if prev is not None:
            for w in writer_insts:
                add_dep_helper(_mi(w), _mi(prev), True, "slot-reuse guard")
        st[0] += 1

    def end(self, tag, bufs, last_inst):
        st = self.state.setdefault(tag, [0, 0, {}])
        st[2][st[1] % bufs] = last_inst
        st[1] += 1


def _pad4(t):
    """[p, (slot, padded r, padded c)] view of a [128, 2*PSP] tile."""
    return t[:].rearrange("p (s r c) -> p s r c", s=2, c=PW)


def _zero_borders(nc, t):
    tr = _pad4(t)
    ins = []
    for s in range(2):
        ins.append(nc.vector.memset(tr[:, s, 0:1, :], 0.0))
        ins.append(nc.vector.memset(tr[:, s, PW - 1:PW, :], 0.0))
        ins.append(nc.vector.memset(tr[:, s, 1:PW - 1, 0:1], 0.0))
        ins.append(nc.vector.memset(tr[:, s, 1:PW - 1, PW - 1:PW], 0.0))
    return ins


def build_kernel(tc, aps, npc, v):
    nc = tc.nc
    kt = v // 128            # K-tiles for pooling
    n_chunk = 512            # pooling column chunk
    nchunks = CHW // n_chunk
    ngroups = npc // 4       # 4-node groups

    feats_pool = aps["feats_pool"]
    feats_shard = aps["feats_shard"]
    a_lhsT = aps["a_lhsT"]
    wconv = aps["wconv"]
    biases = aps["biases"]
    out = aps["out"]

    guard = _SlotGuard()

    with (
        tc.tile_pool(name="persist", bufs=1) as persist,
        tc.tile_pool(name="psum", bufs=4, space="PSUM") as psum_pool,
    ):
        # ---- persistent SBUF state ----
        wsb = persist.tile([128, WCOLS], BF16, tag="wsb")
        pooled = persist.tile([128, 2 * CHW], BF16, tag="pooled")
        bias_sb = persist.tile([128, 6], F32, tag="bias_sb")
        nc.sync.dma_start(wsb[:], wconv[:, :])
        nc.sync.dma_start(bias_sb[:], biases[:, :])

        # ================= stage 1: pooling =================
        with (
            tc.tile_pool(name="asb", bufs=1) as asb_pool,
            tc.tile_pool(name="fstage", bufs=4) as fstage,
        ):
            a_sb = asb_pool.tile([128, kt * 2 * npc], BF16)
            nc.sync.dma_start(a_sb[:], a_lhsT[:, :])
            for cc in range(nchunks):
                fs = fstage.tile([128, kt * n_chunk], BF16, tag="fs")
                dq = nc.sync if cc % 2 == 0 else nc.scalar
                d = dq.dma_start(
                    fs[:], feats_pool[cc * 128:(cc + 1) * 128, :])
                guard.begin("fs", 4, [d])
                last_mm = None
                fs_r = fs[:].rearrange("p (k n) -> p k n", k=kt)
                a_r = a_sb[:].rearrange("p (k m) -> p k m", k=kt)
                for m in range(2):
                    pp = psum_pool.tile([128, n_chunk], F32,
                                        tag=("psP" if m == 0 else "psQ"))
                    for k in range(kt):
                        last_mm = nc.tensor.matmul(
                            pp[:npc, :],
                            a_r[:, k, m * npc:(m + 1) * npc],
                            fs_r[:, k, :],
                            start=(k == 0),
                            stop=(k == kt - 1),
                        )
                    nc.vector.tensor_copy(
                        pooled[:npc, m * CHW + cc * n_chunk:
                               m * CHW + (cc + 1) * n_chunk],
                        pp[:npc, :],
                    )
                guard.end("fs", 4, last_mm)

        # ================= stage 2: conv encoder =================
        # group g = nodes 4g..4g+3: (rg, slot) -> node:
        #   (0, s0)=4g+0  (64, s0)=4g+1  (0, s1)=4g+2  (64, s1)=4g+3
        with (
            tc.tile_pool(name="stg", bufs=3) as stpool,
            tc.tile_pool(name="xt", bufs=6) as xpool,
            tc.tile_pool(name="ht", bufs=4) as hpool,
            tc.tile_pool(name="tmp", bufs=4) as tmppool,
            tc.tile_pool(name="ot", bufs=3) as opool,
            tc.tile_pool(name="fin", bufs=3) as finpool,
            tc.tile_pool(name="nrm", bufs=4) as nrm,
        ):
            ctx = {
                "nc": nc, "wsb": wsb, "bias_sb": bias_sb,
                "psum": psum_pool, "tmp": tmppool, "guard": guard,
            }

            def assemble(g):
                """Stage nodes 4g..4g+3 into a padded x tile."""
                st = stpool.tile([128, 2 * SP], BF16, tag="stg")
                wrts = []
                for ni, (rb, s, n) in enumerate(
                        ((0, 0, 4 * g), (64, 0, 4 * g + 1),
                         (0, 1, 4 * g + 2), (64, 1, 4 * g + 3))):
                    dq = nc.gpsimd if ni % 2 == 0 else nc.sync
                    dst0 = st[rb:rb + C, s * SP:(s + 1) * SP]
                    wrts.append(dq.dma_start(
                        dst0,
                        feats_shard[n:n + 1, :].rearrange(
                            "o (c z) -> (o c) z", c=C),
                    ))
                    for m in range(2):
                        wrts.append(dq.dma_start(
                            st[rb + C * (m + 1):rb + C * (m + 2),
                               s * SP:(s + 1) * SP],
                            pooled[n:n + 1, m * CHW:(m + 1) * CHW],
                        ))
                guard.begin("stg", 3, wrts)

                x = xpool.tile([128, 2 * PSP], BF16, tag="x")
                bz = _zero_borders(nc, x)
                guard.begin("x", 6, bz)
                xr = _pad4(x)
                str_ = st[:].rearrange("p (s r c) -> p s r c", s=2, c=H)
                cps = []
                for s in range(2):
                    cps.append(nc.vector.tensor_copy(
                        xr[0:112, s, 1:1 + H, 1:1 + H], str_[0:112, s]))
                guard.end("stg", 3, cps[-1])
                return x

            def emit48(src_t, layer, evac):
                """One 48->48 3x3 conv for 4 nodes on 64x64 PE tiles.
                evac(hf, pP, pQ) -> last instructions."""
                base = _LAYER_OFF[layer]
                srcr = _pad4(src_t)
                lasts = []
                for hf in range(2):
                    pP = psum_pool.tile([128, 512], F32, tag="psP")
                    pQ = psum_pool.tile([128, 512], F32, tag="psQ")
                    for t, (ky, kx) in enumerate(TAPS):
                        st_, sp_ = (t == 0), (t == 8)
                        wo = base + t * C3
                        for rb, s, pp, ob in ((0, 0, pP, 0), (64, 0, pQ, 0),
                                              (0, 1, pP, 64), (64, 1, pQ, 64)):
                            nc.tensor.matmul(
                                pp[ob:ob + C3, :],
                                wsb[rb:rb + C3, wo:wo + C3],
                                srcr[rb:rb + C3, s,
                                     hf * 16 + ky:hf * 16 + ky + 16,
                                     kx:kx + H],
                                start=st_, stop=sp_,
                                skip_group_check=True,
                                tile_position=(rb, ob),
                            )
                    lasts.extend(evac(hf, pP, pQ))
                return lasts

            def relu_evac(dst_t, bias_col):
                dr = _pad4(dst_t)

                def evac(hf, pP, pQ):
                    res = []
                    for s, pp in ((0, pP), (1, pQ)):
                        res.append(nc.scalar.activation(
                            dr[0:112, s, 1 + hf * 16:1 + hf * 16 + 16,
                               1:1 + H],
                            pp[0:112, :], AF.Relu,
                            bias=bias_sb[0:112, bias_col:bias_col + 1],
                        ))
                    return res
                return evac

            def resid_evac(x_t, bias_col):
                xr = _pad4(x_t)

                def evac(hf, pP, pQ):
                    res = []
                    for s, pp in ((0, pP), (1, pQ)):
                        tmp = tmppool.tile([128, 512], BF16, tag="tmp")
                        a = nc.scalar.activation(
                            tmp[0:112, :], pp[0:112, :], AF.Identity,
                            bias=bias_sb[0:112, bias_col:bias_col + 1],
                        )
                        guard.begin("tmp", 4, [a])
                        win = xr[0:112, s, 1 + hf * 16:1 + hf * 16 + 16,
                                 1:1 + H]
                        t = nc.vector.tensor_add(
                            win, win,
                            tmp[0:112, :].rearrange("p (r c) -> p r c", c=H))
                        guard.end("tmp", 4, t)
                        res.append(t)
                    return res
                return evac

            def emit_final(x0, x1, g0, g1):
                """48->16 conv for groups g0,g1 on 64x32 PE tiles (8 nodes),
                then instance-norm + relu + output DMA."""
                base = _LAYER_OFF[4]
                ot1 = opool.tile([128, SP], F32, tag="ot")
                ot2 = opool.tile([128, SP], F32, tag="ot")
                evs = []
                last_mm = None
                for hf in range(2):
                    pP = psum_pool.tile([128, 512], F32, tag="psP")
                    pQ = psum_pool.tile([128, 512], F32, tag="psQ")
                    tiles = []
                    for gi, xt in ((0, x0), (1, x1)):
                        for rb, s, pp, cp in (
                                (0, 0, pP, 64 * gi),
                                (64, 0, pQ, 64 * gi),
                                (0, 1, pP, 64 * gi + 32),
                                (64, 1, pQ, 64 * gi + 32)):
                            tiles.append((xt, rb, s, pp, cp))
                    for t, (ky, kx) in enumerate(TAPS):
                        st_, sp_ = (t == 0), (t == 8)
                        wo = base + t * C
                        for xt, rb, s, pp, cp in tiles:
                            last_mm = nc.tensor.matmul(
                                pp[cp:cp + C, :],
                                wsb[rb:rb + C3, wo:wo + C],
                                _pad4(xt)[rb:rb + C3, s,
                                          hf * 16 + ky:hf * 16 + ky + 16,
                                          kx:kx + H],
                                start=st_, stop=sp_,
                                skip_group_check=True,
                                tile_position=(rb, cp),
                            )
                    for ot, pp in ((ot1, pP), (ot2, pQ)):
                        e = nc.scalar.activation(
                            ot[0:112, hf * 512:(hf + 1) * 512],
                            pp[0:112, :], AF.Identity,
                            bias=bias_sb[0:112, 2:3])
                        evs.append(e)
                # instance norm + relu + store; ot1 holds nodes
                # (4g0, 4g0+2, 4g1, 4g1+2), ot2 the odd ones.
                for oi, (ot, n_off) in enumerate(((ot1, 0), (ot2, 1))):
                    guard.begin("ot", 3, [evs[oi], evs[oi + 2]])
                    stats = nrm.tile([128, 12], F32, tag="stats")
                    mv = nrm.tile([128, 2], F32, tag="mv")
                    sc = nrm.tile([128, 3], F32, tag="sc")
                    nc.vector.bn_stats(stats[0:112, 0:6], ot[0:112, 0:512])
                    nc.vector.bn_stats(stats[0:112, 6:12], ot[0:112, 512:1024])
                    nc.vector.bn_aggr(mv[0:112, :], stats[0:112, :])
                    nc.scalar.activation(sc[0:112, 0:1], mv[0:112, 1:2],
                                         AF.Sqrt, bias=bias_sb[0:112, 3:4])
                    nc.vector.reciprocal(sc[0:112, 1:2], sc[0:112, 0:1])
                    nc.vector.tensor_scalar(
                        sc[0:112, 2:3], mv[0:112, 0:1], sc[0:112, 1:2], -1.0,
                        op0=mybir.AluOpType.mult, op1=mybir.AluOpType.mult,
                    )
                    fin = finpool.tile([128, SP], F32, tag="fin")
                    ap_i = nc.scalar.activation(
                        fin[0:112, :], ot[0:112, :], AF.Relu,
                        bias=sc[0:112, 2:3], scale=sc[0:112, 1:2])
                    guard.begin("fin", 3, [ap_i])
                    guard.end("ot", 3, ap_i)
                    dmas = []
                    for (p0, p1), n in zip(
                            _B16, (4 * g0, 4 * g0 + 2, 4 * g1, 4 * g1 + 2)):
                        dmas.append(nc.sync.dma_start(
                            out[n + n_off:n + n_off + 1, :].rearrange(
                                "o (c z) -> (o c) z", c=C),
                            fin[p0:p1, :]))
                    guard.end("fin", 3, dmas[-1])
                return last_mm

            # ---- main conv loop: groups in pairs for PE pipelining.
            # Pair p-1's final conv is emitted between pair p's L1a and
            # L1b so its PSUM-evac dependencies are long resolved and the
            # L1a->L1b evac latency is hidden behind it.
            pend = None          # (x0, x1, g0, g1) awaiting final conv
            for gp in range(ngroups // 2):
                g0, g1 = 2 * gp, 2 * gp + 1
                xs = {}
                hs = {}
                for g in (g0, g1):
                    xs[g] = assemble(g)
                for g in (g0, g1):
                    h = hpool.tile([128, 2 * PSP], BF16, tag="h")
                    bz = _zero_borders(nc, h)
                    guard.begin("h", 4, bz)
                    hs[g] = h
                    emit48(xs[g], 0, relu_evac(h, 0))
                if pend is not None:
                    last_mm = emit_final(*pend)
                    guard.end("x", 6, last_mm)
                    guard.end("x", 6, last_mm)
                for g in (g0, g1):
                    lasts = emit48(hs[g], 1, resid_evac(xs[g], 4))
                    guard.end("h", 4, lasts[-1])
                for g in (g0, g1):
                    h2 = hpool.tile([128, 2 * PSP], BF16, tag="h")
                    bz = _zero_borders(nc, h2)
                    guard.begin("h", 4, bz)
                    hs[g] = h2
                    emit48(xs[g], 2, relu_evac(h2, 1))
                for g in (g0, g1):
                    lasts = emit48(hs[g], 3, resid_evac(xs[g], 5))
                    guard.end("h", 4, lasts[-1])
                pend = (xs[g0], xs[g1], g0, g1)
            last_mm = emit_final(*pend)
            guard.end("x", 6, last_mm)
            guard.end("x", 6, last_mm)


# ======================= host side =======================

def _prep_weights(w_list):
    """Pack conv weights into the [128, WCOLS] bf16 lhsT array (two
    partition-group copies at rows 0-47 and 64-111)."""
    wsb = np.zeros((128, WCOLS), np.float32)
    for layer, w in enumerate(w_list):
        co = _LAYER_COUT[layer]
        base = _LAYER_OFF[layer]
        for t, (ky, kx) in enumerate(TAPS):
            lt = np.ascontiguousarray(w[:, :, ky, kx].T)  # [C_in, C_out]
            off = base + t * co
            wsb[0:C3, off:off + co] = lt
            wsb[64:64 + C3, off:off + co] = lt
    import ml_dtypes
    return wsb.astype(ml_dtypes.bfloat16)


def _prep_biases(b1a, b2a, bf, b1b, b2b):
    bias = np.zeros((128, 6), np.float32)
    for p0, p1 in _B48:
        bias[p0:p1, 0] = b1a
        bias[p0:p1, 1] = b2a
        bias[p0:p1, 4] = b1b
        bias[p0:p1, 5] = b2b
    for p0, p1 in _B16:
        bias[p0:p1, 2] = bf
    bias[:, 3] = EPS
    return bias


def _build_adjacency(edges, v):
    src, lab, dst = edges[:, 0], edges[:, 1], edges[:, 2]
    a = np.zeros((2, v, v), np.float32)
    for mi, mask in enumerate((lab > 0, lab < 0)):
        s, d = src[mask], dst[mask]
        np.add.at(a[mi], (d, s), 1.0)
        np.add.at(a[mi], (s, d), 1.0)
    return a


@functools.lru_cache(maxsize=2)
def _build_module(npc, v, ncores):
    nc = bacc.Bacc(
        "TRN2", target_bir_lowering=False, debug=False,
        enable_asserts=False, num_devices=ncores,
    )
    aps = {
        "feats_pool": nc.dram_tensor("feats_pool", [(CHW // 512) * 128,
                                     (v // 128) * 512], BF16,
                                     kind="ExternalInput").ap(),
        "feats_shard": nc.dram_tensor("feats_shard", [npc, CHW], BF16,
                                      kind="ExternalInput").ap(),
        "a_lhsT": nc.dram_tensor("a_lhsT", [128, (v // 128) * 2 * npc], BF16,
                                 kind="ExternalInput").ap(),
        "wconv": nc.dram_tensor("wconv", [128, WCOLS], BF16,
                                kind="ExternalInput").ap(),
        "biases": nc.dram_tensor("biases", [128, 6], F32,
                                 kind="ExternalInput").ap(),
        "out": nc.dram_tensor("out", [npc, CHW], F32,
                              kind="ExternalOutput").ap(),
    }
    with tile.TileContext(nc) as tc:
        build_kernel(tc, aps, npc, v)
    nc.compile()
    return nc


def make_in_maps(feats, edges, w1a, b1a, w1b, b1b, w2a, b2a, w2b, b2b,
                 wf, bf, ncores=NCORES, v=V):
    feats = np.ascontiguousarray(np.asarray(feats, np.float32)).reshape(v, CHW)
    edges = np.asarray(edges)
    npc = v // ncores
    a = _build_adjacency(edges, v)
    wsb = _prep_weights([np.asarray(w) for w in (w1a, w1b, w2a, w2b, wf)])
    bias = _prep_biases(np.asarray(b1a), np.asarray(b2a), np.asarray(bf),
                        np.asarray(b1b), np.asarray(b2b))
    in_maps = []
    for i in range(ncores):
        rows = slice(i * npc, (i + 1) * npc)
        a_sel = np.concatenate([a[0, rows], a[1, rows]], axis=0)  # [2*npc, V]
        import ml_dtypes
        kt = v // 128
        nch = CHW // 512
        fp = feats.reshape(kt, 128, nch, 512).transpose(2, 1, 0, 3)
        fp = np.ascontiguousarray(fp).reshape(nch * 128, kt * 512)
        alt = a_sel.T.reshape(kt, 128, 2 * npc).transpose(1, 0, 2)
        alt = np.ascontiguousarray(alt).reshape(128, kt * 2 * npc)
        in_maps.append({
            "feats_pool": fp.astype(ml_dtypes.bfloat16),
            "feats_shard": np.ascontiguousarray(feats[rows]).astype(
                ml_dtypes.bfloat16),
            "a_lhsT": alt.astype(ml_dtypes.bfloat16),
            "wconv": wsb,
            "biases": bias,
        })
    return in_maps


def run(inputs, trace=False):
    in_maps = make_in_maps(**inputs)
    nc = _build_module(NPC, V, NCORES)
    res = bass_utils.run_bass_kernel_spmd(
        nc, in_maps, core_ids=list(range(NCORES)), trace=trace,
    )
    out = np.concatenate(
        [res.results[i]["out"] for i in range(NCORES)], axis=0
    ).reshape(V, C, H, H)
    return out, res


def kernel(**inputs):
    out, _ = run(inputs, trace=False)
    return out
